# revision 12
# baseline (speedup 1.0000x reference)
"""MoD router kernel for 8 Trainium2 NeuronCores.

Full inputs: x [4, 8192, 1024] f32, w_router [1024] f32, w_block [1024, 1024] f32.
out[b, l] = gelu_tanh(x[b, l] @ w_block) if l in topk(x[b] @ w_router, k=6144)
            else x[b, l]
(top-k membership is all that matters: the reference scatters processed rows
back to their own positions.)

Sharding: core c <- batch row c//2, contiguous half c%2 of L (4096 tokens).

Structure (skew-tolerant: the pairwise score AllGather is triggered ~60us into
each core's own timeline but consumed only ~40us later, so cross-core launch
skew up to ~40us costs nothing):
  phase A: w_block loads first (SDMA round-robins all queued DMAs, so small w
           transfers finish early); w casts on DVE. x streamed in 8 chained
           2MiB DMAs -- the chaining forces in-order completion so chunk j
           lands at ~6(j+1)us and the GEMM pipeline starts at ~10us instead of
           waiting for the whole 20MiB to drain. Scores per tile split 2/2
           between DVE and gpsimd (mult+reduce each) to track the load rate.
  phase B: score exchange. sc_in store + sc_out readback are the LAST entries
           of the SP queue and the collective trigger heads the gpsimd
           dependent chain, so no independent instruction ever queues behind a
           semaphore wait on a strict-FIFO engine.
  phase C: per-tile cast (ACT) -> PE transpose -> xt copy (ACT) -> 16 bf16
           matmuls (K=1024, fp32 PSUM) -> tanh-gelu (ACT) -> bulk store
           (issued from ACT's HWDGE ring, keeping SP free for the collective).
           Vector queue stays [w casts, scores..., search...] so the search
           starts the moment AllGather data lands.
  phase D: 16-ary threshold search, 8 rounds (resolution 3e-8 over +-64 range;
           grid points recomputed bit-identically so the count invariant gives
           the exact top-k membership).
  phase E: mask + absolute-row scatter offsets (p + 128*tile + sel*2^30).
  phase F: fixup: 4-tile-grouped indirect scatters overwrite pass-through rows
           with the resident f32 x rows (selected rows get OOB offsets ->
           skipped); each group depends only on its 4 stores, so the scatters
           overlap the tail of the main loop.
"""
import sys

if "/opt/trn_rl_repo" not in sys.path:
    sys.path.insert(0, "/opt/trn_rl_repo")

from contextlib import ExitStack

import numpy as np

import concourse.bass as bass
import concourse.tile as tile
from concourse import bacc, mybir
from concourse.bass_utils import run_bass_kernel_spmd
from concourse.masks import make_identity
from concourse import bass_isa

dt = mybir.dt
AF = mybir.ActivationFunctionType
ALU = mybir.AluOpType

P = 128
B, L, D = 4, 8192, 1024
TLOC = L // 2          # tokens per core
NT = TLOC // P         # 32 t-tiles per core
DC = D // P            # 8 contraction chunks
K_SEL = int(L * 0.75)  # 6144
N_ROUNDS = 8           # 128 * 16^-8 = 3e-8 resolution (boundary gaps >> this)
SCORE_BOUND = 64.0
LOAD_CHUNK = 4         # t-tiles per load DMA (2 MiB)
FIX_GROUP = 4          # t-tiles per fixup scatter group

_cached = {}


def build_kernel():
    nc = bacc.Bacc("TRN2", target_bir_lowering=False, debug=False, num_devices=8)
    x_d = nc.dram_tensor("x", [TLOC, D], dt.float32, kind="ExternalInput")
    wr_d = nc.dram_tensor("w_router", [D], dt.float32, kind="ExternalInput")
    wb_d = nc.dram_tensor("w_block", [D, D], dt.float32, kind="ExternalInput")
    out_d = nc.dram_tensor("out", [TLOC, D], dt.float32, kind="ExternalOutput")
    sc_in = nc.dram_tensor("sc_in", [TLOC], dt.float32, kind="Internal")
    sc_out = nc.dram_tensor("sc_out", [L], dt.float32, kind="Internal")

    with tile.TileContext(nc) as tc, ExitStack() as ctx:
        const = ctx.enter_context(tc.tile_pool(name="const", bufs=1))
        xpool = ctx.enter_context(tc.tile_pool(name="xn", bufs=1))
        wpool = ctx.enter_context(tc.tile_pool(name="wb", bufs=1))
        xbfp = ctx.enter_context(tc.tile_pool(name="xbf", bufs=2))
        xtp = ctx.enter_context(tc.tile_pool(name="xt", bufs=3))
        yp = ctx.enter_context(tc.tile_pool(name="y", bufs=3))
        smalls = ctx.enter_context(tc.tile_pool(name="smalls", bufs=1))
        psx = ctx.enter_context(tc.tile_pool(name="psx", bufs=2, space="PSUM"))
        psy = ctx.enter_context(tc.tile_pool(name="psy", bufs=3, space="PSUM"))

        # ---- constants ----
        ident = const.tile([P, P], dt.bfloat16)
        make_identity(nc, ident[:])
        ones_row = const.tile([1, P], dt.float32)
        nc.vector.memset(ones_row[:], 1.0)

        # ---- score / search tiles ----
        scores_loc = smalls.tile([P, NT], dt.float32)
        scores_full = smalls.tile([P, 2 * NT], dt.float32)
        ge3 = smalls.tile([P, 15, 2 * NT], dt.float32)
        cnts = smalls.tile([P, 15], dt.float32)
        gk = smalls.tile([P, 15], dt.float32)
        tcand = smalls.tile([P, 15], dt.float32)
        jrow_i = smalls.tile([P, 15], dt.int32)
        jrow = smalls.tile([P, 15], dt.float32)
        lo = smalls.tile([P, 1], dt.float32)
        m = smalls.tile([P, 1], dt.float32)
        msel = smalls.tile([P, NT], dt.float32)
        pcol_i = smalls.tile([P, 1], dt.int32)
        pcol = smalls.tile([P, 1], dt.float32)
        offs_f = smalls.tile([P, NT], dt.float32)
        offs = smalls.tile([P, NT], dt.int32)
        tmp = smalls.tile([P, D], dt.float32)
        tmpg = smalls.tile([P, D], dt.float32)
        cnts_red = smalls.tile([P, 15], dt.float32)

        # iotas early on gpsimd (before anything AG-dependent in its queue)
        nc.gpsimd.iota(jrow_i[:], pattern=[[1, 15]], base=1,
                       channel_multiplier=0)
        nc.gpsimd.iota(pcol_i[:], pattern=[[0, 1]], base=0,
                       channel_multiplier=1)

        # ---- phase A: w_block first (small DMAs drain early), casts on DVE --
        w_sb = wpool.tile([P, DC, D], dt.bfloat16)
        for c in range(DC):
            wstage = wpool.tile([P, D], dt.float32, tag="wstage", bufs=4)
            nc.sync.dma_start(wstage[:], wb_d.ap()[c * P:(c + 1) * P, :])
            nc.vector.tensor_copy(out=w_sb[:, c, :], in_=wstage[:])

        # w_router -> broadcast over all partitions via K=1 matmuls
        wr_sb = const.tile([1, D], dt.float32)
        nc.sync.dma_start(wr_sb[:], wr_d.ap())
        w_rep = const.tile([P, D], dt.float32)
        for h in range(2):
            sl = slice(h * 512, (h + 1) * 512)
            pm = psy.tile([P, D], dt.float32, tag="psy")
            nc.tensor.matmul(pm[:, :512], ones_row[:], wr_sb[:, sl],
                             start=True, stop=True)
            nc.vector.tensor_copy(w_rep[:, sl], pm[:, :512])

        # x streamed in depth-2-chained DMAs: two transfers in flight
        # overlap each other's completion latency while still completing
        # roughly in order (true streaming); a small first chunk gets the
        # GEMM pipeline going early
        xn_all = xpool.tile([P, NT, D], dt.float32)
        CHUNKS = [2, 4, 4, 4, 4, 4, 4, 4, 2]
        lds = []
        a = 0
        for cn in CHUNKS:
            with nc.named_scope("load"):
                ld = nc.sync.dma_start(
                    xn_all[:, a:a + cn, :],
                    x_d.ap()[a * P:(a + cn) * P, :].rearrange(
                        "(c p) d -> p c d", p=P))
                if len(lds) >= 2:
                    tile.add_dep_helper(ld.ins, lds[-2].ins,
                                        reason="stream x chunks, depth 2")
                lds.append(ld)
            with nc.named_scope("scores"):
                for i in range(a, a + cn):
                    # DVE mults half the tiles (and reduces all); gpsimd
                    # pre-multiplies the other half
                    if i % 2 == 0:
                        nc.vector.tensor_tensor(out=tmp[:],
                                                in0=xn_all[:, i, :],
                                                in1=w_rep[:], op=ALU.mult)
                        nc.vector.reduce_sum(scores_loc[:, i:i + 1], tmp[:],
                                             axis=mybir.AxisListType.X)
                    else:
                        nc.gpsimd.tensor_tensor(out=tmpg[:],
                                                in0=xn_all[:, i, :],
                                                in1=w_rep[:], op=ALU.mult)
                        nc.vector.reduce_sum(scores_loc[:, i:i + 1], tmpg[:],
                                             axis=mybir.AxisListType.X)
            a += cn

        # ---- phase B: pairwise score exchange (SP queue tail + gpsimd) ----
        with nc.named_scope("coll"):
            nc.sync.dma_start(sc_in.ap(), scores_loc[:])
            nc.gpsimd.collective_compute(
                "AllGather", ALU.bypass,
                ins=[sc_in.ap()], outs=[sc_out.ap()],
                replica_groups=[[0, 1], [2, 3], [4, 5], [6, 7]])
            nc.sync.dma_start(scores_full[:], sc_out.ap())

        # ---- phase C: main compute loop ----
        store_insts = []
        for i in range(NT):
            with nc.named_scope("cast"):
                xbf = xbfp.tile([P, D], dt.bfloat16, tag="xbf")
                nc.scalar.copy(xbf[:], xn_all[:, i, :])
            xt = xtp.tile([P, DC, P], dt.bfloat16, tag="xt")
            px = psx.tile([P, DC, P], dt.bfloat16, tag="psx")
            with nc.named_scope("xpose"):
                for c in range(DC):
                    nc.tensor.transpose(px[:, c, :], xbf[:, c * P:(c + 1) * P],
                                        ident[:])
                nc.scalar.copy(xt[:], px[:])
            y = yp.tile([P, D], dt.float32, tag="y")
            py = psy.tile([P, D], dt.float32, tag="psy")
            with nc.named_scope("gemm"):
                for h in range(2):
                    for c in range(DC):
                        nc.tensor.matmul(
                            py[:, h * 512:(h + 1) * 512], xt[:, c, :],
                            w_sb[:, c, h * 512:(h + 1) * 512],
                            start=(c == 0), stop=(c == DC - 1))
            with nc.named_scope("gelu"):
                nc.scalar.activation(y[:], py[:], AF.Gelu_apprx_tanh)
            with nc.named_scope("store"):
                st = nc.scalar.dma_start(out_d.ap()[i * P:(i + 1) * P, :],
                                         y[:])
            store_insts.append(st)

        # ---- phase D: threshold search (DVE + gpsimd partition reduce) ----
        with nc.named_scope("search"):
            nc.vector.tensor_copy(out=jrow[:], in_=jrow_i[:])
            nc.vector.memset(lo[:], -SCORE_BOUND)
            sc_b = scores_full[:].rearrange("p (a x) -> p a x", a=1) \
                .to_broadcast([P, 15, 2 * NT])
            t_b = tcand[:].rearrange("p (j x) -> p j x", x=1) \
                .to_broadcast([P, 15, 2 * NT])
            for r in range(N_ROUNDS):
                w16 = 2.0 * SCORE_BOUND / (16.0 ** (r + 1))
                # tcand[:, j] = lo + (j+1)*w16  (dyadic, exact fp32)
                nc.vector.tensor_scalar(out=tcand[:], in0=jrow[:],
                                        scalar1=w16, scalar2=lo[:],
                                        op0=ALU.mult, op1=ALU.add)
                nc.vector.tensor_tensor(out=ge3[:], in0=sc_b, in1=t_b,
                                        op=ALU.is_ge)
                nc.vector.reduce_sum(cnts[:], ge3[:],
                                     axis=mybir.AxisListType.X)
                nc.gpsimd.partition_all_reduce(
                    cnts_red[:], cnts[:], P, bass_isa.ReduceOp.add)
                # gk = (count >= k); m = #intervals passed (row-sum)
                nc.vector.tensor_scalar(out=gk[:], in0=cnts_red[:],
                                        scalar1=float(K_SEL), scalar2=None,
                                        op0=ALU.is_ge)
                nc.vector.reduce_sum(m[:], gk[:],
                                     axis=mybir.AxisListType.X)
                # lo += m*w16 (bit-identical to the compared grid point)
                nc.vector.tensor_scalar(out=lo[:], in0=m[:],
                                        scalar1=w16, scalar2=lo[:],
                                        op0=ALU.mult, op1=ALU.add)

        # ---- phase E: mask + per-tile scatter offsets ----
        with nc.named_scope("mask"):
            # selected = score >= thr(=lo); offs = p + sel*2^30 (per-tile)
            nc.vector.tensor_scalar(out=msel[:], in0=scores_loc[:],
                                    scalar1=lo[:], scalar2=None,
                                    op0=ALU.is_ge)
            nc.vector.tensor_copy(out=pcol[:], in_=pcol_i[:])
            nc.vector.tensor_scalar(out=offs_f[:], in0=msel[:],
                                    scalar1=float(2 ** 30),
                                    scalar2=pcol[:],
                                    op0=ALU.mult, op1=ALU.add)
            nc.vector.tensor_copy(out=offs[:], in_=offs_f[:])

        # ---- phase F: overwrite pass-through rows with resident x rows ----
        # (selected rows get OOB offsets -> skipped); per-tile scatters, each
        # depending only on its tile's store, so they overlap the loop tail
        with nc.named_scope("fixup"):
            for i in range(NT):
                sl = out_d.ap()[i * P:(i + 1) * P, :]
                sl_rel = bass.AP(tensor=sl.tensor, offset=0, ap=sl.ap,
                                 dep_tracking_offset=i * P * D)
                fx = nc.gpsimd.indirect_dma_start(
                    out=sl_rel,
                    out_offset=bass.IndirectOffsetOnAxis(ap=offs[:, i:i + 1],
                                                         axis=0),
                    in_=xn_all[:, i, :],
                    in_offset=None,
                    element_offset=i * P * D,
                    bounds_check=P - 1,
                    oob_is_err=False,
                )
                tile.add_dep_helper(fx.ins, store_insts[i].ins,
                                    reason="fixup scatter after bulk y store")

    nc.compile()
    return nc


def _get_nc():
    if "nc" not in _cached:
        _cached["nc"] = build_kernel()
    return _cached["nc"]


def run(x, w_router, w_block, trace=False, trace_kwargs=None):
    nc = _get_nc()
    x = np.ascontiguousarray(x, dtype=np.float32)
    w_router = np.ascontiguousarray(w_router, dtype=np.float32)
    w_block = np.ascontiguousarray(w_block, dtype=np.float32)
    in_maps = []
    for c in range(8):
        b, h = c // 2, c % 2
        in_maps.append({
            "x": x[b, h * TLOC:(h + 1) * TLOC, :],
            "w_router": w_router,
            "w_block": w_block,
        })
    res = run_bass_kernel_spmd(nc, in_maps, core_ids=list(range(8)),
                               trace=trace, **(trace_kwargs or {}))
    out = np.empty((B, L, D), dtype=np.float32)
    for c in range(8):
        b, h = c // 2, c % 2
        out[b, h * TLOC:(h + 1) * TLOC, :] = res.results[c]["out"]
    return out, res


def kernel(x, w_router, w_block):
    out, _ = run(x, w_router, w_block, trace=False)
    return out


# revision 15
# speedup vs baseline: 1.0723x; 1.0723x over previous
"""MoD router kernel for 8 Trainium2 NeuronCores.

Full inputs: x [4, 8192, 1024] f32, w_router [1024] f32, w_block [1024, 1024] f32.
out[b, l] = gelu_tanh(x[b, l] @ w_block) if l in topk(x[b] @ w_router, k=6144)
            else x[b, l]
(top-k membership is all that matters: the reference scatters processed rows
back to their own positions.)

Sharding: core c <- batch row c//2, contiguous half c%2 of L (4096 tokens).

Structure (skew-tolerant: the pairwise score AllGather is triggered ~60us into
each core's own timeline but consumed only ~40us later, so cross-core launch
skew up to ~40us costs nothing):
  phase A: w_block loads first (SDMA round-robins all queued DMAs, so small w
           transfers finish early); w casts on DVE. x streamed in 8 chained
           2MiB DMAs -- the chaining forces in-order completion so chunk j
           lands at ~6(j+1)us and the GEMM pipeline starts at ~10us instead of
           waiting for the whole 20MiB to drain. Scores per tile split 2/2
           between DVE and gpsimd (mult+reduce each) to track the load rate.
  phase B: score exchange. sc_in store + sc_out readback are the LAST entries
           of the SP queue and the collective trigger heads the gpsimd
           dependent chain, so no independent instruction ever queues behind a
           semaphore wait on a strict-FIFO engine.
  phase C: per-tile cast (ACT) -> PE transpose -> xt copy (ACT) -> 16 bf16
           matmuls (K=1024, fp32 PSUM) -> tanh-gelu (ACT) -> bulk store
           (issued from ACT's HWDGE ring, keeping SP free for the collective).
           Vector queue stays [w casts, scores..., search...] so the search
           starts the moment AllGather data lands.
  phase D: 16-ary threshold search, 8 rounds (resolution 3e-8 over +-64 range;
           grid points recomputed bit-identically so the count invariant gives
           the exact top-k membership).
  phase E: mask + absolute-row scatter offsets (p + 128*tile + sel*2^30).
  phase F: fixup: 4-tile-grouped indirect scatters overwrite pass-through rows
           with the resident f32 x rows (selected rows get OOB offsets ->
           skipped); each group depends only on its 4 stores, so the scatters
           overlap the tail of the main loop.
"""
import sys

if "/opt/trn_rl_repo" not in sys.path:
    sys.path.insert(0, "/opt/trn_rl_repo")

from contextlib import ExitStack

import numpy as np

import concourse.bass as bass
import concourse.tile as tile
from concourse import bacc, mybir
from concourse.bass_utils import run_bass_kernel_spmd
from concourse.masks import make_identity
from concourse import bass_isa

dt = mybir.dt
AF = mybir.ActivationFunctionType
ALU = mybir.AluOpType

P = 128
B, L, D = 4, 8192, 1024
TLOC = L // 2          # tokens per core
NT = TLOC // P         # 32 t-tiles per core
DC = D // P            # 8 contraction chunks
K_SEL = int(L * 0.75)  # 6144
N_ROUNDS = 8           # 128 * 16^-8 = 3e-8 resolution (boundary gaps >> this)
SCORE_BOUND = 64.0
LOAD_CHUNK = 4         # t-tiles per load DMA (2 MiB)
S_MERGE = 27           # tail tiles: DVE-merge + SP-store instead of scatter

_cached = {}


def build_kernel():
    nc = bacc.Bacc("TRN2", target_bir_lowering=False, debug=False, num_devices=8)
    x_d = nc.dram_tensor("x", [TLOC, D], dt.float32, kind="ExternalInput")
    wr_d = nc.dram_tensor("w_router", [D], dt.float32, kind="ExternalInput")
    wb_d = nc.dram_tensor("w_block", [D, D], dt.float32, kind="ExternalInput")
    out_d = nc.dram_tensor("out", [TLOC, D], dt.float32, kind="ExternalOutput")
    sc_in = nc.dram_tensor("sc_in", [TLOC], dt.float32, kind="Internal")
    sc_out = nc.dram_tensor("sc_out", [L], dt.float32, kind="Internal")

    with tile.TileContext(nc) as tc, ExitStack() as ctx:
        const = ctx.enter_context(tc.tile_pool(name="const", bufs=1))
        xpool = ctx.enter_context(tc.tile_pool(name="xn", bufs=1))
        wpool = ctx.enter_context(tc.tile_pool(name="wb", bufs=1))
        xbfp = ctx.enter_context(tc.tile_pool(name="xbf", bufs=2))
        xtp = ctx.enter_context(tc.tile_pool(name="xt", bufs=2))
        yp = ctx.enter_context(tc.tile_pool(name="y", bufs=5))
        smalls = ctx.enter_context(tc.tile_pool(name="smalls", bufs=1))
        psx = ctx.enter_context(tc.tile_pool(name="psx", bufs=2, space="PSUM"))
        psy = ctx.enter_context(tc.tile_pool(name="psy", bufs=3, space="PSUM"))

        # ---- constants ----
        ident = const.tile([P, P], dt.bfloat16)
        make_identity(nc, ident[:])
        ones_row = const.tile([1, P], dt.float32)
        nc.vector.memset(ones_row[:], 1.0)

        # ---- score / search tiles ----
        scores_loc = smalls.tile([P, NT], dt.float32)
        scores_full = smalls.tile([P, 2 * NT], dt.float32)
        ge3 = smalls.tile([P, 15, 2 * NT], dt.float32)
        cnts = smalls.tile([P, 15], dt.float32)
        gk = smalls.tile([P, 15], dt.float32)
        tcand = smalls.tile([P, 15], dt.float32)
        jrow_i = smalls.tile([P, 15], dt.int32)
        jrow = smalls.tile([P, 15], dt.float32)
        lo = smalls.tile([P, 1], dt.float32)
        m = smalls.tile([P, 1], dt.float32)
        msel = smalls.tile([P, NT], dt.float32)
        minv = smalls.tile([P, NT], dt.float32)
        minv_i = smalls.tile([P, NT], dt.int8)
        pcol_i = smalls.tile([P, 1], dt.int32)
        pcol = smalls.tile([P, 1], dt.float32)
        offs_f = smalls.tile([P, NT], dt.float32)
        offs = smalls.tile([P, NT], dt.int32)
        tmp = smalls.tile([P, D], dt.float32)
        tmpg = smalls.tile([P, D], dt.float32)
        tmpg2 = smalls.tile([P, D], dt.float32)
        cnts_red = smalls.tile([P, 15], dt.float32)

        # iotas early on gpsimd (before anything AG-dependent in its queue)
        nc.gpsimd.iota(jrow_i[:], pattern=[[1, 15]], base=1,
                       channel_multiplier=0)
        nc.gpsimd.iota(pcol_i[:], pattern=[[0, 1]], base=0,
                       channel_multiplier=1)

        # ---- phase A: w_block first (alone at full bandwidth), casts on DVE -
        wr_sb = const.tile([1, D], dt.float32)
        nc.sync.dma_start(wr_sb[:], wr_d.ap())
        w_sb = wpool.tile([P, DC, D], dt.bfloat16)
        for c in range(DC):
            wstage = wpool.tile([P, D], dt.float32, tag="wstage", bufs=2)
            nc.sync.dma_start(wstage[:], wb_d.ap()[c * P:(c + 1) * P, :])
            nc.vector.tensor_copy(out=w_sb[:, c, :], in_=wstage[:])

        # w_router -> broadcast over all partitions via K=1 matmuls
        w_rep = const.tile([P, D], dt.float32)
        for h in range(2):
            sl = slice(h * 512, (h + 1) * 512)
            pm = psy.tile([P, D], dt.float32, tag="psy")
            nc.tensor.matmul(pm[:, :512], ones_row[:], wr_sb[:, sl],
                             start=True, stop=True)
            nc.vector.tensor_copy(w_rep[:, sl], pm[:, :512])

        # x streamed in depth-2-chained DMAs: two transfers in flight
        # overlap each other's completion latency while still completing
        # roughly in order (true streaming); a small first chunk gets the
        # GEMM pipeline going early
        xn_all = xpool.tile([P, NT, D], dt.float32)
        CHUNKS = [2, 2, 4, 4, 4, 4, 4, 4, 4]
        lds = []
        a = 0
        for cn in CHUNKS:
            with nc.named_scope("load"):
                ld = nc.sync.dma_start(
                    xn_all[:, a:a + cn, :],
                    x_d.ap()[a * P:(a + cn) * P, :].rearrange(
                        "(c p) d -> p c d", p=P))
                # depth-2 chain: two loads in flight, in-order-ish completion
                if len(lds) in (1, 2):
                    tile.add_dep_helper(ld.ins, lds[0].ins,
                                        reason="stream x chunks, depth 2")
                elif len(lds) >= 3:
                    tile.add_dep_helper(ld.ins, lds[-2].ins,
                                        reason="stream x chunks, depth 2")
                lds.append(ld)
            with nc.named_scope("scores"):
                for i in range(a, a + cn):
                    # DVE mults half the tiles (and reduces all); gpsimd
                    # pre-multiplies the other half (double-buffered so the
                    # DVE reduce never WAR-blocks the next gpsimd mult)
                    if i % 2 == 0:
                        nc.vector.tensor_tensor(out=tmp[:],
                                                in0=xn_all[:, i, :],
                                                in1=w_rep[:], op=ALU.mult)
                        nc.vector.reduce_sum(scores_loc[:, i:i + 1], tmp[:],
                                             axis=mybir.AxisListType.X)
                    else:
                        buf = tmpg if (i // 2) % 2 == 0 else tmpg2
                        nc.gpsimd.tensor_tensor(out=buf[:],
                                                in0=xn_all[:, i, :],
                                                in1=w_rep[:], op=ALU.mult)
                        nc.vector.reduce_sum(scores_loc[:, i:i + 1], buf[:],
                                             axis=mybir.AxisListType.X)
            a += cn

        # ---- phase B: pairwise score exchange (SP queue tail + gpsimd) ----
        with nc.named_scope("coll"):
            nc.sync.dma_start(sc_in.ap(), scores_loc[:])
            nc.gpsimd.collective_compute(
                "AllGather", ALU.bypass,
                ins=[sc_in.ap()], outs=[sc_out.ap()],
                replica_groups=[[0, 1], [2, 3], [4, 5], [6, 7]])
            nc.sync.dma_start(scores_full[:], sc_out.ap())

        # ---- phase C: main compute loop ----
        store_insts = []
        y_tail = []
        for i in range(NT):
            with nc.named_scope("cast"):
                xbf = xbfp.tile([P, D], dt.bfloat16, tag="xbf")
                nc.scalar.copy(xbf[:], xn_all[:, i, :])
            xt = xtp.tile([P, DC, P], dt.bfloat16, tag="xt")
            px = psx.tile([P, DC, P], dt.bfloat16, tag="psx")
            with nc.named_scope("xpose"):
                for c in range(DC):
                    nc.tensor.transpose(px[:, c, :], xbf[:, c * P:(c + 1) * P],
                                        ident[:])
                nc.scalar.copy(xt[:], px[:])
            y = yp.tile([P, D], dt.float32, tag="y")
            py = psy.tile([P, D], dt.float32, tag="psy")
            with nc.named_scope("gemm"):
                for h in range(2):
                    for c in range(DC):
                        nc.tensor.matmul(
                            py[:, h * 512:(h + 1) * 512], xt[:, c, :],
                            w_sb[:, c, h * 512:(h + 1) * 512],
                            start=(c == 0), stop=(c == DC - 1))
            with nc.named_scope("gelu"):
                nc.scalar.activation(y[:], py[:], AF.Gelu_apprx_tanh)
            if i < S_MERGE:
                with nc.named_scope("store"):
                    st = nc.scalar.dma_start(out_d.ap()[i * P:(i + 1) * P, :],
                                             y[:])
                store_insts.append(st)
            else:
                y_tail.append(y)

        # ---- phase D: threshold search (DVE + gpsimd partition reduce) ----
        with nc.named_scope("search"):
            nc.vector.tensor_copy(out=jrow[:], in_=jrow_i[:])
            nc.vector.memset(lo[:], -SCORE_BOUND)
            sc_b = scores_full[:].rearrange("p (a x) -> p a x", a=1) \
                .to_broadcast([P, 15, 2 * NT])
            t_b = tcand[:].rearrange("p (j x) -> p j x", x=1) \
                .to_broadcast([P, 15, 2 * NT])
            for r in range(N_ROUNDS):
                w16 = 2.0 * SCORE_BOUND / (16.0 ** (r + 1))
                # tcand[:, j] = lo + (j+1)*w16  (dyadic, exact fp32)
                nc.vector.tensor_scalar(out=tcand[:], in0=jrow[:],
                                        scalar1=w16, scalar2=lo[:],
                                        op0=ALU.mult, op1=ALU.add)
                nc.vector.tensor_tensor(out=ge3[:], in0=sc_b, in1=t_b,
                                        op=ALU.is_ge)
                nc.vector.reduce_sum(cnts[:], ge3[:],
                                     axis=mybir.AxisListType.X)
                nc.gpsimd.partition_all_reduce(
                    cnts_red[:], cnts[:], P, bass_isa.ReduceOp.add)
                # gk = (count >= k); m = #intervals passed (row-sum)
                nc.vector.tensor_scalar(out=gk[:], in0=cnts_red[:],
                                        scalar1=float(K_SEL), scalar2=None,
                                        op0=ALU.is_ge)
                nc.vector.reduce_sum(m[:], gk[:],
                                     axis=mybir.AxisListType.X)
                # lo += m*w16 (bit-identical to the compared grid point)
                nc.vector.tensor_scalar(out=lo[:], in0=m[:],
                                        scalar1=w16, scalar2=lo[:],
                                        op0=ALU.mult, op1=ALU.add)

        # ---- phase E: mask + per-tile scatter offsets ----
        with nc.named_scope("mask"):
            # selected = score >= thr(=lo); offs = p + sel*2^30 (per-tile)
            nc.vector.tensor_scalar(out=msel[:], in0=scores_loc[:],
                                    scalar1=lo[:], scalar2=None,
                                    op0=ALU.is_ge)
            nc.vector.tensor_copy(out=pcol[:], in_=pcol_i[:])
            nc.vector.tensor_scalar(out=offs_f[:], in0=msel[:],
                                    scalar1=float(2 ** 30),
                                    scalar2=pcol[:],
                                    op0=ALU.mult, op1=ALU.add)
            nc.vector.tensor_copy(out=offs[:], in_=offs_f[:])
            # inv = 1 - msel (pass-through rows), for the tail merges
            nc.vector.tensor_scalar(out=minv[:], in0=msel[:],
                                    scalar1=-1.0, scalar2=1.0,
                                    op0=ALU.mult, op1=ALU.add)
            nc.vector.tensor_copy(out=minv_i[:], in_=minv[:])

        # ---- phase F: overwrite pass-through rows with resident x rows ----
        # (selected rows get OOB offsets -> skipped); per-tile scatters, each
        # depending only on its tile's store, so they overlap the loop tail
        with nc.named_scope("merge"):
            for i in range(S_MERGE, NT):
                y = y_tail[i - S_MERGE]
                nc.vector.copy_predicated(
                    y[:], minv_i[:, i:i + 1].to_broadcast([P, D]),
                    xn_all[:, i, :])
                nc.sync.dma_start(out_d.ap()[i * P:(i + 1) * P, :], y[:])
        with nc.named_scope("fixup"):
            for i in range(S_MERGE):
                sl = out_d.ap()[i * P:(i + 1) * P, :]
                sl_rel = bass.AP(tensor=sl.tensor, offset=0, ap=sl.ap,
                                 dep_tracking_offset=i * P * D)
                fx = nc.gpsimd.indirect_dma_start(
                    out=sl_rel,
                    out_offset=bass.IndirectOffsetOnAxis(ap=offs[:, i:i + 1],
                                                         axis=0),
                    in_=xn_all[:, i, :],
                    in_offset=None,
                    element_offset=i * P * D,
                    bounds_check=P - 1,
                    oob_is_err=False,
                )
                tile.add_dep_helper(fx.ins, store_insts[i].ins,
                                    reason="fixup scatter after bulk y store")

    nc.compile()
    return nc


def _get_nc():
    if "nc" not in _cached:
        _cached["nc"] = build_kernel()
    return _cached["nc"]


def run(x, w_router, w_block, trace=False, trace_kwargs=None):
    nc = _get_nc()
    x = np.ascontiguousarray(x, dtype=np.float32)
    w_router = np.ascontiguousarray(w_router, dtype=np.float32)
    w_block = np.ascontiguousarray(w_block, dtype=np.float32)
    in_maps = []
    for c in range(8):
        b, h = c // 2, c % 2
        in_maps.append({
            "x": x[b, h * TLOC:(h + 1) * TLOC, :],
            "w_router": w_router,
            "w_block": w_block,
        })
    res = run_bass_kernel_spmd(nc, in_maps, core_ids=list(range(8)),
                               trace=trace, **(trace_kwargs or {}))
    out = np.empty((B, L, D), dtype=np.float32)
    for c in range(8):
        b, h = c // 2, c % 2
        out[b, h * TLOC:(h + 1) * TLOC, :] = res.results[c]["out"]
    return out, res


def kernel(x, w_router, w_block):
    out, _ = run(x, w_router, w_block, trace=False)
    return out
